# revision 10
# baseline (speedup 1.0000x reference)
"""Signature-kernel Gram matrix on 8 NeuronCores.

Math (per pair of sequences x (128,8), y (128,8)):
  K = exp(x@y.T - 0.5|x|^2 - 0.5|y|^2)            (RBF gram, sigma=1)
  diff = second mixed finite difference of K       (127,127)
  Goursat PDE grid, dyadic order 1 (2x2 fine refinement):
    G[i,j] = c1*(G[i-1,j]+G[i,j-1]) - c2*G[i-1,j-1]
    c1 = 1 + diff/8 + diff^2/192,  c2 = 1 - diff^2/192
  answer = G[254,254]

Pipeline (per core: 2 xs * 16 ys = 32 pairs):

Phase A: two batched input DMAs (ys->[128,16*8], xs->[128,2*8]); per-seq
  PE transposes into AUG [8, 18*128] pipelined through PSUM with the
  norm matmuls software-pipelined two sequences behind; x norms become
  per-partition bias columns for the phase-B exp.

Phase B (software-pipelined front/back per 4-pair group, work spread
  over PE/ACT/DVE/Pool): gram matmul + y-norm rank-1 into PSUM, exp
  with x-norm bias (ACT), column diff (Pool), row shift (PE matmul by
  shifted identity), mixed diff (DVE), then the scan coefficients
    a1 = -c1/c2, b1 = -c2
  stored as fp16 DEVIATIONS (1+a1, 1+b1): deviations are O(diff/8), so
  fp16 costs ~5e-4 relative on the deviation (~2e-5 absolute on the
  coefficient) and halves all staging traffic. Stores to DRAM pair-major
  ride inside the loop; skewed per-chunk reloads (early rows first, so
  phase E's first multiplier windows unblock ahead of the bulk) fill
  A1X/B1X [128, 260*32] fp16, partition = 4*pair + chunk, with chunk
  skew SK=2 baked into the column offsets. Pad regions get two small
  edge memsets; the loads overwrite everything in between.

Phase E executes the row recurrence as a 4-chunk wavefront, ALL on DVE
  (260 macro steps, one fine row per step per chunk, chunks skewed by
  2 rows). Per step:
  - stream_shuffle: scan(t-2)'s last column (still intact in the buffer
    scan(t) is about to overwrite) shifted down one chunk within each
    32-partition quadrant (partition = 4*pair + chunk makes the chunk
    relay quadrant-local), written into col 1;
  - copy_predicated forces col 1 = 1.0 on chunk-0 partitions (grid
    boundary);
  - interleave copy builds d0 = (prv_j, prv_{j-1}) pairs from the other
    buffer via a negative-stride view (col 1 doubles as the j=-1
    partner, v1's dual use);
  - one two-slot tensor_tensor_scan computes the whole row chunk:
    s' = (prv_j + y_{j-1})*a1_j ; y_j = (prv_{j-1} + s')*b1_j,
    initial = col 1.
  Every phase-E dependency lives on the DVE semaphore, so no blocking
  EventSemaphore parks: the steady-state step is ~511ns (engine-chain
  limited). GPSIMD expands fp16 deviation windows into f32 multipliers
  (fused -1.0) every 16 steps, double-buffered 4 deep.

Sharding: data-parallel over batch_x: core c owns x rows {2c, 2c+1} x
all 16 ys = 32 pairs. Host gathers the (16,16) output.
"""

import numpy as np
from contextlib import ExitStack

import concourse.bass as bass
import concourse.bacc as bacc
import concourse.tile as tile
from concourse import mybir
from concourse.bass_utils import run_bass_kernel_spmd

F32 = mybir.dt.float32
F16 = mybir.dt.float16
AL = mybir.AluOpType
AF = mybir.ActivationFunctionType

N_CORES = 8
L = 128          # sequence length
D = 8            # feature dim
NY = 16          # all ys per core
NX = 2           # xs per core
NP = NX * NY     # 32 pairs per core
M = L - 1        # 127 coarse grid
MP = 128         # padded coarse columns
G = 254          # fine grid rows (dyadic order 1)
NCH = 4          # phase-E chunks (partition groups of 32)
CW = 64          # fine cols per chunk
CC = CW // 2     # coarse cols per chunk
SK = 2           # row skew between adjacent chunks
NT = G + SK * (NCH - 1)   # macro steps (260)
WS = 16          # multiplier-window size in macro steps
BG = 4           # pairs per phase-B group
NSEQ = NX + NY


def _rep2(ap):
    """[P, n] -> [P, n, 2] zero-stride repeat view."""
    return bass.AP(tensor=ap.tensor, offset=ap.offset,
                   ap=[ap.ap[0], ap.ap[1], [0, 2]])


def _build():
    nc = bacc.Bacc()
    xs_t = nc.dram_tensor("xs", [NX * L, D], F32, kind="ExternalInput")
    ys_t = nc.dram_tensor("ys", [NY * L, D], F32, kind="ExternalInput")
    idn_t = nc.dram_tensor("idn", [L, L], F32, kind="ExternalInput")
    shf_t = nc.dram_tensor("shf", [L, L], F32, kind="ExternalInput")
    msk_t = nc.dram_tensor("msk", [L, 1], mybir.dt.int32, kind="ExternalInput")
    out_t = nc.dram_tensor("out", [NP, 1], F32, kind="ExternalOutput")

    with ExitStack() as ctx:
        tc = ctx.enter_context(tile.TileContext(nc))
        constp = ctx.enter_context(tc.tile_pool(name="constp", bufs=1))
        iop = ctx.enter_context(tc.tile_pool(name="iop", bufs=2))
        psp = ctx.enter_context(tc.tile_pool(name="psp", bufs=1, space="PSUM"))
        sqp = ctx.enter_context(tc.tile_pool(name="sqp", bufs=4))
        workp = ctx.enter_context(tc.tile_pool(name="workp", bufs=3))
        cbp = ctx.enter_context(tc.tile_pool(name="cbp", bufs=3))
        fbp = ctx.enter_context(tc.tile_pool(name="fbp", bufs=3))
        xp = ctx.enter_context(tc.tile_pool(name="xp", bufs=1))
        abp = ctx.enter_context(tc.tile_pool(name="abp", bufs=1))
        rowp = ctx.enter_context(tc.tile_pool(name="rowp", bufs=1))
        dramp = ctx.enter_context(tc.tile_pool(name="dramp", bufs=1, space="DRAM"))

        # ---- constants ----
        idn_s = iop.tile([L, L], F32, tag="idn_s", bufs=1)
        nc.sync.dma_start(out=idn_s, in_=idn_t[:, :])
        idn = constp.tile([L, L], F32)
        nc.vector.tensor_copy(idn, idn_s)
        ones8 = constp.tile([D, 1], F32)
        nc.vector.memset(ones8, 1.0)
        onecol = constp.tile([L, 1], F32)
        nc.vector.memset(onecol, 1.0)
        ONE = constp.tile([1, L], F32)
        nc.vector.memset(ONE, 1.0)
        ONEB = constp.tile([1, BG * L], F32)
        nc.vector.memset(ONEB, 1.0)

        # ---- Phase A: batched loads, pipelined transposes + norms ----
        yr_s = iop.tile([L, NY * D], F32, tag="yr_s", bufs=1)
        nc.sync.dma_start(
            out=yr_s,
            in_=bass.AP(tensor=ys_t, offset=0,
                        ap=[[D, L], [L * D, NY], [1, D]]),
        )
        YR = constp.tile([L, NY * D], F32)
        nc.vector.tensor_copy(YR, yr_s)
        xr_s = iop.tile([L, NX * D], F32, tag="xr_s", bufs=1)
        nc.sync.dma_start(
            out=xr_s,
            in_=bass.AP(tensor=xs_t, offset=0,
                        ap=[[D, L], [L * D, NX], [1, D]]),
        )
        XR = constp.tile([L, NX * D], F32)
        nc.vector.tensor_copy(XR, xr_s)
        shf_s = iop.tile([L, L], F32, tag="shf_s", bufs=1)
        nc.sync.dma_start(out=shf_s, in_=shf_t[:, :])
        shf = constp.tile([L, L], F32)
        nc.vector.tensor_copy(shf, shf_s)
        msk_s = iop.tile([L, 1], mybir.dt.int32, tag="msk_s", bufs=1)
        nc.sync.dma_start(out=msk_s, in_=msk_t[:, :])
        MASKC = constp.tile([L, 1], mybir.dt.int32)
        nc.vector.tensor_copy(MASKC, msk_s)

        AUGX = constp.tile([D, NX * L], F32)
        AUGY0 = constp.tile([D, 4 * L], F32)
        AUGY1 = constp.tile([D, 4 * L], F32)
        AUGY2 = constp.tile([D, 4 * L], F32)
        AUGY3 = constp.tile([D, 4 * L], F32)
        AUGY = [AUGY0, AUGY1, AUGY2, AUGY3]
        NRMY0 = constp.tile([1, 4 * L], F32)
        NRMY1 = constp.tile([1, 4 * L], F32)
        NRMY2 = constp.tile([1, 4 * L], F32)
        NRMY3 = constp.tile([1, 4 * L], F32)
        NRMY = [NRMY0, NRMY1, NRMY2, NRMY3]
        # per-partition x norms: XNC[t, i] = -0.5 * |x_i[t]|^2
        SQX = constp.tile([L, NX * D], F32)
        nc.scalar.square(SQX, XR)
        XNR = constp.tile([L, NX], F32)
        nc.vector.tensor_reduce(
            XNR.rearrange("p (a b) -> p a b", b=1),
            SQX.rearrange("p (a b) -> p a b", b=D),
            mybir.AxisListType.X, AL.add)
        XNC = constp.tile([L, NX], F32)
        nc.vector.tensor_scalar_mul(XNC, XNR, -0.5)
        nrm_ps = None
        sq_tiles = {}

        def a_front(s):
            if s < NY:
                rawv = YR[:, s * D:(s + 1) * D]
            else:
                rawv = XR[:, (s - NY) * D:(s - NY + 1) * D]
            pst = psp.tile([D, L], F32, tag="pst", bufs=2)
            nc.tensor.transpose(pst, rawv, idn)
            if s < NY:
                dstt = AUGY[s // 4][0:D, (s % 4) * L:(s % 4 + 1) * L]
            else:
                dstt = AUGX[0:D, (s - NY) * L:(s - NY + 1) * L]
            nc.vector.tensor_copy(dstt, pst)
            if s < NY:
                sq = sqp.tile([D, L], F32, tag="sq")
                nc.scalar.square(sq, pst)
                sq_tiles[s] = sq

        def a_back(s):
            nonlocal nrm_ps
            sq = sq_tiles.pop(s)
            if s % 4 == 0:
                nrm_ps = psp.tile([1, 4 * L], F32, tag="nrm", bufs=1)
            nc.tensor.matmul(nrm_ps[:, (s % 4) * L:(s % 4 + 1) * L],
                             ones8, sq, start=True, stop=True)
            if s % 4 == 3:
                nc.vector.tensor_scalar_mul(
                    NRMY[s // 4][0:1, :], nrm_ps[:, :], -0.5)

        # b_front/b_back defined below; A and B interleave group-wise
        PENDING_BF = []

        # ---- A1X/B1X fp16 deviation stores, skew-region zero padding ----
        a1d = dramp.tile([NP, M * MP], F16, tag="a1d")
        b1d = dramp.tile([NP, M * MP], F16, tag="b1d")
        A1X = xp.tile([128, NT * CC], F16)
        B1X = xp.tile([128, NT * CC], F16)
        # pad zeroing: loads cover cols [2k*CC, (2k+2M+2)*CC) per chunk k;
        # every col in [6CC, 256CC) is load-covered on all partitions, so only
        # the global lead/tail margins need explicit zeros (loads overwrite).
        LEAD = 2 * (NCH - 1) * CC
        for T in (A1X, B1X):
            nc.vector.memset(T[:, 0:LEAD], 0.0)
            nc.vector.memset(T[:, (2 * M + 2) * CC:NT * CC], 0.0)

        # ---- Phase B: coefficients per group of 4 pairs + stores ----
        # software-pipelined emission: front(g+1) is emitted before back(g)
        # so no engine queue head ever waits on a freshly-issued cross-engine
        # producer (convoy breaker).
        fronts = {}

        def b_front(g):
            p0 = g * BG
            iloc, j0 = p0 // NY, p0 % NY
            kps = psp.tile([L, BG * L], F32, tag="kps", bufs=3)
            nc.tensor.matmul(kps, AUGX[:, iloc * L:(iloc + 1) * L],
                             AUGY[j0 // 4][:, :], start=True, stop=False)
            nc.tensor.matmul(kps, ONE, NRMY[j0 // 4][:, :], start=False, stop=True)
            kex = workp.tile([L, BG * L], F32, tag="kex")
            nc.scalar.activation(kex, kps, AF.Exp,
                                 bias=XNC[:, iloc:iloc + 1], scale=1.0)
            kv = kex.rearrange("p (a b) -> p a b", b=L)
            db = workp.tile([L, BG * M], F32, tag="db")
            dbv = db.rearrange("p (a b) -> p a b", b=M)
            nc.gpsimd.tensor_sub(dbv, kv[:, :, 1:L], kv[:, :, 0:M])
            dbs = psp.tile([L, BG * M], F32, tag="dbs", bufs=2)
            nc.tensor.matmul(dbs, shf, db)
            DIFFB = cbp.tile([M, BG * MP], F32, tag="df")
            nc.vector.memset(
                DIFFB.rearrange("p (a b) -> p a b", b=MP)[:, :, M:M + 1], 0.0)
            dfv = DIFFB.rearrange("p (a b) -> p a b", b=MP)
            nc.vector.tensor_sub(
                dfv[:, :, 0:M],
                dbs[0:M, :].rearrange("p (a b) -> p a b", b=M),
                db[0:M, :].rearrange("p (a b) -> p a b", b=M),
            )
            fronts[g] = DIFFB

        def b_back(g):
            DIFFB = fronts.pop(g)
            # QB = DIFFB^2 ; T1 = QB/192 + 1 ; c1 = DIFFB/8 + T1
            # b1 = T1 - 2 = -c2 ; N = c1 + b1 = c1 - c2
            # a1p = 1 + a1 = N / b1 ; b1p = 1 + b1 = T1 - 1
            QB = cbp.tile([M, BG * MP], F32, tag="qb")
            nc.gpsimd.tensor_mul(QB, DIFFB, DIFFB)
            T1 = cbp.tile([M, BG * MP], F32, tag="t1")
            nc.scalar.activation(T1, QB, AF.Copy, bias=1.0, scale=1.0 / 192.0)
            c1c = cbp.tile([M, BG * MP], F32, tag="c1")
            nc.vector.scalar_tensor_tensor(c1c, DIFFB, 0.125, T1,
                                           AL.mult, AL.add)
            b1c = cbp.tile([M, BG * MP], F32, tag="b1")
            nc.scalar.activation(b1c, T1, AF.Copy, bias=-2.0, scale=1.0)
            ib2 = cbp.tile([M, BG * MP], F32, tag="ib")
            nc.vector.reciprocal(out=ib2, in_=b1c)
            a1m = cbp.tile([M, BG * MP], F32, tag="am")
            nc.vector.tensor_mul(a1m, c1c, ib2)
            a1p = fbp.tile([M, BG * MP], F16, tag="ap")
            nc.vector.tensor_scalar_add(a1p, a1m, 1.0)
            b1p = fbp.tile([M, BG * MP], F16, tag="bp")
            nc.scalar.activation(b1p, T1, AF.Copy, bias=-1.0, scale=1.0)
            # store fp16 deviations to DRAM pair-major (hidden under B)
            for dr, sb in ((a1d, a1p), (b1d, b1p)):
                drv = dr.rearrange("p (i j) -> i p j", j=MP)
                nc.sync.dma_start(
                    out=drv[:, g * BG:(g + 1) * BG, :],
                    in_=sb.rearrange("p (a b) -> p a b", b=MP),
                )

        NG = NP // BG
        seq_order = [NY, NY + 1] + list(range(NY))
        for idx in range(NSEQ + 4):
            if idx < NSEQ:
                a_front(seq_order[idx])
            if 4 <= idx < NY + 4:
                s = idx - 4
                a_back(s)
                if s % 4 == 3:
                    b_front(s // 4)
        emit = []
        for gg in range(4, NG):
            emit += [("back", gg - 4), ("front", gg)]
        emit += [("back", g) for g in range(NG - 4, NG)]
        for kind, g in emit:
            if kind == "front":
                b_front(g)
            else:
                b_back(g)

        # ---- Phase D: skewed per-chunk fp16 reloads ----
        # early rows (ic<E0) first so phase E's first windows unblock before
        # the bulk transfers finish
        E0 = 24
        xpitch = A1X.ap[0][0]
        for ic0, icn in ((0, E0), (E0, M - E0)):
            for src_d, dst in ((a1d, A1X), (b1d, B1X)):
                for k in range(NCH):
                    for dlt in range(2):
                        out_ap = bass.AP(
                            tensor=dst.tensor,
                            offset=dst.offset + k * xpitch
                            + CC * (SK * k + dlt) + ic0 * 2 * CC,
                            ap=[[4 * xpitch, NP], [2 * CC, icn], [1, CC]],
                        )
                        sv = src_d[:, :]
                        in_ap = bass.AP(
                            tensor=sv.tensor,
                            offset=sv.offset + CC * k + ic0 * MP,
                            ap=[sv.ap[0], [MP, icn], [1, CC]],
                        )
                        eng = nc.sync if (k % 2 == 0) else nc.scalar
                        eng.dma_start(out=out_ap, in_=in_ap)

        # ---- Phase E: all-DVE wavefront ----
        # partition = 4*pair + chunk. Per step, four DVE ops:
        #   stream_shuffle: scan(t-2)'s last column (col 2CW+1 of the buffer
        #     scan(t) will overwrite) shifted down one chunk within each
        #     quadrant, written into that buffer's col 1
        #   copy_predicated: forces col 1 = 1.0 on chunk-0 partitions
        #   interleave copy (d0) from the other buffer
        #   two-slot scan with initial = col 1 (same value later serves as
        #     copy(t+1)'s pair partner, v1's dual use)
        # Every phase-E dependency lives on the DVE semaphore: no
        # EventSemaphore parks, no PE/ACT involvement.
        s0t = rowp.tile([128, 2 * CW + 2], F32, tag="s0")
        s1t = rowp.tile([128, 2 * CW + 2], F32, tag="s1")
        S = [s0t, s1t]
        nc.vector.memset(S[0][:, :], 1.0)
        nc.vector.memset(S[1][:, :], 1.0)

        SHMASK = [i if i % 4 == 0 else i - 1 for i in range(32)]

        ABW = None
        for t in range(NT):
            if t % WS == 0:
                w0, w1 = t, min(NT, t + WS)
                n = w1 - w0
                ABW = abp.tile([128, WS * 2 * CW], F32, tag="abw", bufs=4)
                av = bass.AP(
                    tensor=ABW.tensor, offset=ABW.offset,
                    ap=[ABW.ap[0], [4, n * CC], [2, 2]],
                )
                bv = bass.AP(
                    tensor=ABW.tensor, offset=ABW.offset + 1,
                    ap=[ABW.ap[0], [4, n * CC], [2, 2]],
                )
                nc.gpsimd.tensor_scalar_add(
                    av, _rep2(A1X[:, CC * w0:CC * w1]), -1.0)
                nc.gpsimd.tensor_scalar_add(
                    bv, _rep2(B1X[:, CC * w0:CC * w1]), -1.0)
            sw, sr = S[t % 2], S[(t + 1) % 2]
            nc.vector.stream_shuffle(
                sw[:, 1:2], sw[:, 2 * CW + 1:2 * CW + 2], SHMASK)
            nc.vector.copy_predicated(sw[:, 1:2], MASKC[:, 0:1], onecol[:, 0:1])
            d0 = rowp.tile([128, 2 * CW], F32, tag="d0", bufs=2)
            iv = bass.AP(
                tensor=sr.tensor, offset=sr.offset + 3,
                ap=[sr.ap[0], [2, CW], [-2, 2]],
            )
            nc.vector.tensor_copy(d0.rearrange("p (a b) -> p a b", b=2), iv)
            nc.vector.tensor_tensor_scan(
                sw[:, 2:2 * CW + 2], d0,
                ABW[:, (t % WS) * 2 * CW:(t % WS + 1) * 2 * CW],
                sw[:, 1:2], AL.add, AL.mult,
            )

        # answer: pair p at partition 4p+3 (chunk 3), col 2*61+3 = 125
        fs = S[(NT - 1) % 2]
        spitch = fs.ap[0][0]
        fin_src = bass.AP(
            tensor=fs.tensor,
            offset=fs.offset + 3 * spitch + (2 * 62 + 1),
            ap=[[4 * spitch, NP], [1, 1]],
        )
        nc.sync.dma_start(out=out_t[:, :], in_=fin_src)

    nc.finalize()
    return nc


_CACHE = {}


def _get_nc():
    if "nc" not in _CACHE:
        _CACHE["nc"] = _build()
    return _CACHE["nc"]


def run(xs, ys, trace=False):
    xs = np.ascontiguousarray(np.asarray(xs), dtype=np.float32)
    ys = np.ascontiguousarray(np.asarray(ys), dtype=np.float32)
    assert xs.shape == (16, L, D) and ys.shape == (16, L, D)
    nc = _get_nc()
    idn = np.eye(L, dtype=np.float32)
    shf = np.eye(L, k=-1, dtype=np.float32)
    msk = (np.arange(L) % 4 == 0).astype(np.int32).reshape(L, 1)
    in_maps = []
    for c in range(N_CORES):
        in_maps.append(
            {
                "xs": xs[2 * c:2 * c + 2].reshape(NX * L, D).copy(),
                "ys": ys.reshape(NY * L, D).copy(),
                "idn": idn,
                "shf": shf,
                "msk": msk,
            }
        )
    try:
        res = run_bass_kernel_spmd(nc, in_maps, list(range(N_CORES)), trace=trace)
    except ModuleNotFoundError:
        res = run_bass_kernel_spmd(nc, in_maps, list(range(N_CORES)), trace=False)
    rows = [res.results[c]["out"].reshape(NX, NY) for c in range(N_CORES)]
    out = np.concatenate(rows, axis=0)
    return out, res


def kernel(xs, ys):
    out, _ = run(xs, ys)
    return out


# revision 11
# speedup vs baseline: 1.0018x; 1.0018x over previous
"""Signature-kernel Gram matrix on 8 NeuronCores.

Math (per pair of sequences x (128,8), y (128,8)):
  K = exp(x@y.T - 0.5|x|^2 - 0.5|y|^2)            (RBF gram, sigma=1)
  diff = second mixed finite difference of K       (127,127)
  Goursat PDE grid, dyadic order 1 (2x2 fine refinement):
    G[i,j] = c1*(G[i-1,j]+G[i,j-1]) - c2*G[i-1,j-1]
    c1 = 1 + diff/8 + diff^2/192,  c2 = 1 - diff^2/192
  answer = G[254,254]

Pipeline (per core: 2 xs * 16 ys = 32 pairs):

Phase A: two batched input DMAs (ys->[128,16*8], xs->[128,2*8]); per-seq
  PE transposes into AUG [8, 18*128] pipelined through PSUM with the
  norm matmuls software-pipelined two sequences behind; x norms become
  per-partition bias columns for the phase-B exp.

Phase B (software-pipelined front/back per 4-pair group, work spread
  over PE/ACT/DVE/Pool): gram matmul + y-norm rank-1 into PSUM, exp
  with x-norm bias (ACT), column diff (Pool), row shift (PE matmul by
  shifted identity), mixed diff (DVE), then the scan coefficients
    a1 = -c1/c2, b1 = -c2
  stored as fp16 DEVIATIONS (1+a1, 1+b1): deviations are O(diff/8), so
  fp16 costs ~5e-4 relative on the deviation (~2e-5 absolute on the
  coefficient) and halves all staging traffic. Stores to DRAM pair-major
  ride inside the loop; skewed per-chunk reloads (early rows first, so
  phase E's first multiplier windows unblock ahead of the bulk) fill
  A1X/B1X [128, 260*32] fp16, partition = 4*pair + chunk, with chunk
  skew SK=2 baked into the column offsets. Pad regions get two small
  edge memsets; the loads overwrite everything in between.

Phase E executes the row recurrence as a 4-chunk wavefront, ALL on DVE
  (260 macro steps, one fine row per step per chunk, chunks skewed by
  2 rows). Per step:
  - stream_shuffle: scan(t-2)'s last column (still intact in the buffer
    scan(t) is about to overwrite) shifted down one chunk within each
    32-partition quadrant (partition = 4*pair + chunk makes the chunk
    relay quadrant-local), written into col 1;
  - copy_predicated forces col 1 = 1.0 on chunk-0 partitions (grid
    boundary);
  - interleave copy builds d0 = (prv_j, prv_{j-1}) pairs from the other
    buffer via a negative-stride view (col 1 doubles as the j=-1
    partner, v1's dual use);
  - one two-slot tensor_tensor_scan computes the whole row chunk:
    s' = (prv_j + y_{j-1})*a1_j ; y_j = (prv_{j-1} + s')*b1_j,
    initial = col 1.
  Every phase-E dependency lives on the DVE semaphore, so no blocking
  EventSemaphore parks: the steady-state step is ~511ns (engine-chain
  limited). GPSIMD expands fp16 deviation windows into f32 multipliers
  (fused -1.0) every 16 steps, double-buffered 4 deep.

Sharding: data-parallel over batch_x: core c owns x rows {2c, 2c+1} x
all 16 ys = 32 pairs. Host gathers the (16,16) output.
"""

import numpy as np
from contextlib import ExitStack

import concourse.bass as bass
import concourse.bacc as bacc
import concourse.tile as tile
from concourse import mybir
from concourse.bass_utils import run_bass_kernel_spmd

F32 = mybir.dt.float32
F16 = mybir.dt.float16
AL = mybir.AluOpType
AF = mybir.ActivationFunctionType

N_CORES = 8
L = 128          # sequence length
D = 8            # feature dim
NY = 16          # all ys per core
NX = 2           # xs per core
NP = NX * NY     # 32 pairs per core
M = L - 1        # 127 coarse grid
MP = 128         # padded coarse columns
G = 254          # fine grid rows (dyadic order 1)
NCH = 4          # phase-E chunks (partition groups of 32)
CW = 64          # fine cols per chunk
CC = CW // 2     # coarse cols per chunk
SK = 2           # row skew between adjacent chunks
NT = G + SK * (NCH - 1)   # macro steps (260)
WS = 16          # multiplier-window size in macro steps
BG = 4           # pairs per phase-B group
NSEQ = NX + NY


def _rep2(ap):
    """[P, n] -> [P, n, 2] zero-stride repeat view."""
    return bass.AP(tensor=ap.tensor, offset=ap.offset,
                   ap=[ap.ap[0], ap.ap[1], [0, 2]])


def _build():
    nc = bacc.Bacc()
    xs_t = nc.dram_tensor("xs", [NX * L, D], F32, kind="ExternalInput")
    ys_t = nc.dram_tensor("ys", [NY * L, D], F32, kind="ExternalInput")
    idn_t = nc.dram_tensor("idn", [L, L], F32, kind="ExternalInput")
    shf_t = nc.dram_tensor("shf", [L, L], F32, kind="ExternalInput")
    msk_t = nc.dram_tensor("msk", [L, 1], mybir.dt.int32, kind="ExternalInput")
    out_t = nc.dram_tensor("out", [NP, 1], F32, kind="ExternalOutput")

    with ExitStack() as ctx:
        tc = ctx.enter_context(tile.TileContext(nc))
        constp = ctx.enter_context(tc.tile_pool(name="constp", bufs=1))
        iop = ctx.enter_context(tc.tile_pool(name="iop", bufs=2))
        psp = ctx.enter_context(tc.tile_pool(name="psp", bufs=1, space="PSUM"))
        sqp = ctx.enter_context(tc.tile_pool(name="sqp", bufs=4))
        workp = ctx.enter_context(tc.tile_pool(name="workp", bufs=3))
        cbp = ctx.enter_context(tc.tile_pool(name="cbp", bufs=3))
        fbp = ctx.enter_context(tc.tile_pool(name="fbp", bufs=3))
        xp = ctx.enter_context(tc.tile_pool(name="xp", bufs=1))
        abp = ctx.enter_context(tc.tile_pool(name="abp", bufs=1))
        rowp = ctx.enter_context(tc.tile_pool(name="rowp", bufs=1))
        dramp = ctx.enter_context(tc.tile_pool(name="dramp", bufs=1, space="DRAM"))

        # ---- constants ----
        idn_s = iop.tile([L, L], F32, tag="idn_s", bufs=1)
        nc.sync.dma_start(out=idn_s, in_=idn_t[:, :])
        idn = constp.tile([L, L], F32)
        nc.vector.tensor_copy(idn, idn_s)
        ones8 = constp.tile([D, 1], F32)
        nc.vector.memset(ones8, 1.0)
        onecol = constp.tile([L, 1], F32)
        nc.vector.memset(onecol, 1.0)
        ONE = constp.tile([1, L], F32)
        nc.vector.memset(ONE, 1.0)
        ONEB = constp.tile([1, BG * L], F32)
        nc.vector.memset(ONEB, 1.0)

        # ---- Phase A: batched loads, pipelined transposes + norms ----
        yr_s = iop.tile([L, NY * D], F32, tag="yr_s", bufs=1)
        nc.sync.dma_start(
            out=yr_s,
            in_=bass.AP(tensor=ys_t, offset=0,
                        ap=[[D, L], [L * D, NY], [1, D]]),
        )
        YR = constp.tile([L, NY * D], F32)
        nc.vector.tensor_copy(YR, yr_s)
        xr_s = iop.tile([L, NX * D], F32, tag="xr_s", bufs=1)
        nc.sync.dma_start(
            out=xr_s,
            in_=bass.AP(tensor=xs_t, offset=0,
                        ap=[[D, L], [L * D, NX], [1, D]]),
        )
        XR = constp.tile([L, NX * D], F32)
        nc.vector.tensor_copy(XR, xr_s)
        shf_s = iop.tile([L, L], F32, tag="shf_s", bufs=1)
        nc.sync.dma_start(out=shf_s, in_=shf_t[:, :])
        shf = constp.tile([L, L], F32)
        nc.vector.tensor_copy(shf, shf_s)
        msk_s = iop.tile([L, 1], mybir.dt.int32, tag="msk_s", bufs=1)
        nc.sync.dma_start(out=msk_s, in_=msk_t[:, :])
        MASKC = constp.tile([L, 1], mybir.dt.int32)
        nc.vector.tensor_copy(MASKC, msk_s)

        AUGX = constp.tile([D, NX * L], F32)
        AUGY0 = constp.tile([D, 4 * L], F32)
        AUGY1 = constp.tile([D, 4 * L], F32)
        AUGY2 = constp.tile([D, 4 * L], F32)
        AUGY3 = constp.tile([D, 4 * L], F32)
        AUGY = [AUGY0, AUGY1, AUGY2, AUGY3]
        NRMY0 = constp.tile([1, 4 * L], F32)
        NRMY1 = constp.tile([1, 4 * L], F32)
        NRMY2 = constp.tile([1, 4 * L], F32)
        NRMY3 = constp.tile([1, 4 * L], F32)
        NRMY = [NRMY0, NRMY1, NRMY2, NRMY3]
        # per-partition x norms: XNC[t, i] = -0.5 * |x_i[t]|^2
        SQX = constp.tile([L, NX * D], F32)
        nc.scalar.square(SQX, XR)
        XNR = constp.tile([L, NX], F32)
        nc.vector.tensor_reduce(
            XNR.rearrange("p (a b) -> p a b", b=1),
            SQX.rearrange("p (a b) -> p a b", b=D),
            mybir.AxisListType.X, AL.add)
        XNC = constp.tile([L, NX], F32)
        nc.vector.tensor_scalar_mul(XNC, XNR, -0.5)
        nrm_ps = None
        sq_tiles = {}

        def a_front(s):
            if s < NY:
                rawv = YR[:, s * D:(s + 1) * D]
            else:
                rawv = XR[:, (s - NY) * D:(s - NY + 1) * D]
            pst = psp.tile([D, L], F32, tag="pst", bufs=2)
            nc.tensor.transpose(pst, rawv, idn)
            if s < NY:
                dstt = AUGY[s // 4][0:D, (s % 4) * L:(s % 4 + 1) * L]
            else:
                dstt = AUGX[0:D, (s - NY) * L:(s - NY + 1) * L]
            nc.vector.tensor_copy(dstt, pst)
            if s < NY:
                sq = sqp.tile([D, L], F32, tag="sq")
                nc.scalar.square(sq, pst)
                sq_tiles[s] = sq

        def a_back(s):
            nonlocal nrm_ps
            sq = sq_tiles.pop(s)
            if s % 4 == 0:
                nrm_ps = psp.tile([1, 4 * L], F32, tag="nrm", bufs=1)
            nc.tensor.matmul(nrm_ps[:, (s % 4) * L:(s % 4 + 1) * L],
                             ones8, sq, start=True, stop=True)
            if s % 4 == 3:
                nc.vector.tensor_scalar_mul(
                    NRMY[s // 4][0:1, :], nrm_ps[:, :], -0.5)



        # ---- A1X/B1X fp16 deviation stores, skew-region zero padding ----
        a1d = dramp.tile([NP, M * MP], F16, tag="a1d")
        b1d = dramp.tile([NP, M * MP], F16, tag="b1d")
        A1X = xp.tile([128, NT * CC], F16)
        B1X = xp.tile([128, NT * CC], F16)
        # pad zeroing: loads cover cols [2k*CC, (2k+2M+2)*CC) per chunk k;
        # every col in [6CC, 256CC) is load-covered on all partitions, so only
        # the global lead/tail margins need explicit zeros (loads overwrite).
        LEAD = 2 * (NCH - 1) * CC
        for T in (A1X, B1X):
            nc.vector.memset(T[:, 0:LEAD], 0.0)
            nc.vector.memset(T[:, (2 * M + 2) * CC:NT * CC], 0.0)

        # ---- Phase B: coefficients per group of 4 pairs + stores ----
        # software-pipelined emission: front(g+1) is emitted before back(g)
        # so no engine queue head ever waits on a freshly-issued cross-engine
        # producer (convoy breaker).
        fronts = {}

        def b_front(g):
            p0 = g * BG
            iloc, j0 = p0 // NY, p0 % NY
            kps = psp.tile([L, BG * L], F32, tag="kps", bufs=3)
            nc.tensor.matmul(kps, AUGX[:, iloc * L:(iloc + 1) * L],
                             AUGY[j0 // 4][:, :], start=True, stop=False)
            nc.tensor.matmul(kps, ONE, NRMY[j0 // 4][:, :], start=False, stop=True)
            kex = workp.tile([L, BG * L], F32, tag="kex")
            nc.scalar.activation(kex, kps, AF.Exp,
                                 bias=XNC[:, iloc:iloc + 1], scale=1.0)
            kv = kex.rearrange("p (a b) -> p a b", b=L)
            db = workp.tile([L, BG * M], F32, tag="db")
            dbv = db.rearrange("p (a b) -> p a b", b=M)
            nc.gpsimd.tensor_sub(dbv, kv[:, :, 1:L], kv[:, :, 0:M])
            dbs = psp.tile([L, BG * M], F32, tag="dbs", bufs=2)
            nc.tensor.matmul(dbs, shf, db)
            DIFFB = cbp.tile([M, BG * MP], F32, tag="df")
            nc.vector.memset(
                DIFFB.rearrange("p (a b) -> p a b", b=MP)[:, :, M:M + 1], 0.0)
            dfv = DIFFB.rearrange("p (a b) -> p a b", b=MP)
            nc.vector.tensor_sub(
                dfv[:, :, 0:M],
                dbs[0:M, :].rearrange("p (a b) -> p a b", b=M),
                db[0:M, :].rearrange("p (a b) -> p a b", b=M),
            )
            fronts[g] = DIFFB

        def b_back(g):
            DIFFB = fronts.pop(g)
            # QB = DIFFB^2 ; T1 = QB/192 + 1 ; c1 = DIFFB/8 + T1
            # b1 = T1 - 2 = -c2 ; N = c1 + b1 = c1 - c2
            # a1p = 1 + a1 = N / b1 ; b1p = 1 + b1 = T1 - 1
            QB = cbp.tile([M, BG * MP], F32, tag="qb")
            nc.gpsimd.tensor_mul(QB, DIFFB, DIFFB)
            T1 = cbp.tile([M, BG * MP], F32, tag="t1")
            nc.scalar.activation(T1, QB, AF.Copy, bias=1.0, scale=1.0 / 192.0)
            c1c = cbp.tile([M, BG * MP], F32, tag="c1")
            nc.vector.scalar_tensor_tensor(c1c, DIFFB, 0.125, T1,
                                           AL.mult, AL.add)
            b1c = cbp.tile([M, BG * MP], F32, tag="b1")
            nc.scalar.activation(b1c, T1, AF.Copy, bias=-2.0, scale=1.0)
            ib2 = cbp.tile([M, BG * MP], F32, tag="ib")
            nc.vector.reciprocal(out=ib2, in_=b1c)
            a1m = cbp.tile([M, BG * MP], F32, tag="am")
            nc.vector.tensor_mul(a1m, c1c, ib2)
            a1p = fbp.tile([M, BG * MP], F16, tag="ap")
            nc.vector.tensor_scalar_add(a1p, a1m, 1.0)
            b1p = fbp.tile([M, BG * MP], F16, tag="bp")
            nc.scalar.activation(b1p, T1, AF.Copy, bias=-1.0, scale=1.0)
            # store fp16 deviations to DRAM pair-major (hidden under B)
            for dr, sb in ((a1d, a1p), (b1d, b1p)):
                drv = dr.rearrange("p (i j) -> i p j", j=MP)
                nc.sync.dma_start(
                    out=drv[:, g * BG:(g + 1) * BG, :],
                    in_=sb.rearrange("p (a b) -> p a b", b=MP),
                )

        NG = NP // BG
        seq_order = [NY, NY + 1] + list(range(NY))
        for idx in range(NSEQ + 4):
            if idx < NSEQ:
                a_front(seq_order[idx])
            if 4 <= idx < NY + 4:
                a_back(idx - 4)
        for gg in range(NG + 1):
            if gg < NG:
                b_front(gg)
            if gg >= 1:
                b_back(gg - 1)

        # ---- Phase D: skewed per-chunk fp16 reloads ----
        # early rows (ic<E0) first so phase E's first windows unblock before
        # the bulk transfers finish
        E0 = 24
        xpitch = A1X.ap[0][0]
        for ic0, icn in ((0, E0), (E0, M - E0)):
            for src_d, dst in ((a1d, A1X), (b1d, B1X)):
                for k in range(NCH):
                    for dlt in range(2):
                        out_ap = bass.AP(
                            tensor=dst.tensor,
                            offset=dst.offset + k * xpitch
                            + CC * (SK * k + dlt) + ic0 * 2 * CC,
                            ap=[[4 * xpitch, NP], [2 * CC, icn], [1, CC]],
                        )
                        sv = src_d[:, :]
                        in_ap = bass.AP(
                            tensor=sv.tensor,
                            offset=sv.offset + CC * k + ic0 * MP,
                            ap=[sv.ap[0], [MP, icn], [1, CC]],
                        )
                        eng = nc.sync if (k % 2 == 0) else nc.scalar
                        eng.dma_start(out=out_ap, in_=in_ap)

        # ---- Phase E: all-DVE wavefront ----
        # partition = 4*pair + chunk. Per step, four DVE ops:
        #   stream_shuffle: scan(t-2)'s last column (col 2CW+1 of the buffer
        #     scan(t) will overwrite) shifted down one chunk within each
        #     quadrant, written into that buffer's col 1
        #   copy_predicated: forces col 1 = 1.0 on chunk-0 partitions
        #   interleave copy (d0) from the other buffer
        #   two-slot scan with initial = col 1 (same value later serves as
        #     copy(t+1)'s pair partner, v1's dual use)
        # Every phase-E dependency lives on the DVE semaphore: no
        # EventSemaphore parks, no PE/ACT involvement.
        s0t = rowp.tile([128, 2 * CW + 2], F32, tag="s0")
        s1t = rowp.tile([128, 2 * CW + 2], F32, tag="s1")
        S = [s0t, s1t]
        nc.vector.memset(S[0][:, :], 1.0)
        nc.vector.memset(S[1][:, :], 1.0)

        SHMASK = [i if i % 4 == 0 else i - 1 for i in range(32)]

        ABW = None
        for t in range(NT):
            if t % WS == 0:
                w0, w1 = t, min(NT, t + WS)
                n = w1 - w0
                ABW = abp.tile([128, WS * 2 * CW], F32, tag="abw", bufs=4)
                av = bass.AP(
                    tensor=ABW.tensor, offset=ABW.offset,
                    ap=[ABW.ap[0], [4, n * CC], [2, 2]],
                )
                bv = bass.AP(
                    tensor=ABW.tensor, offset=ABW.offset + 1,
                    ap=[ABW.ap[0], [4, n * CC], [2, 2]],
                )
                nc.gpsimd.tensor_scalar_add(
                    av, _rep2(A1X[:, CC * w0:CC * w1]), -1.0)
                nc.gpsimd.tensor_scalar_add(
                    bv, _rep2(B1X[:, CC * w0:CC * w1]), -1.0)
            sw, sr = S[t % 2], S[(t + 1) % 2]
            nc.vector.stream_shuffle(
                sw[:, 1:2], sw[:, 2 * CW + 1:2 * CW + 2], SHMASK)
            nc.vector.copy_predicated(sw[:, 1:2], MASKC[:, 0:1], onecol[:, 0:1])
            d0 = rowp.tile([128, 2 * CW], F32, tag="d0", bufs=2)
            iv = bass.AP(
                tensor=sr.tensor, offset=sr.offset + 3,
                ap=[sr.ap[0], [2, CW], [-2, 2]],
            )
            nc.vector.tensor_copy(d0.rearrange("p (a b) -> p a b", b=2), iv)
            nc.vector.tensor_tensor_scan(
                sw[:, 2:2 * CW + 2], d0,
                ABW[:, (t % WS) * 2 * CW:(t % WS + 1) * 2 * CW],
                sw[:, 1:2], AL.add, AL.mult,
            )

        # answer: pair p at partition 4p+3 (chunk 3), col 2*61+3 = 125
        fs = S[(NT - 1) % 2]
        spitch = fs.ap[0][0]
        fin_src = bass.AP(
            tensor=fs.tensor,
            offset=fs.offset + 3 * spitch + (2 * 62 + 1),
            ap=[[4 * spitch, NP], [1, 1]],
        )
        nc.sync.dma_start(out=out_t[:, :], in_=fin_src)

    nc.finalize()
    return nc


_CACHE = {}


def _get_nc():
    if "nc" not in _CACHE:
        _CACHE["nc"] = _build()
    return _CACHE["nc"]


def run(xs, ys, trace=False):
    xs = np.ascontiguousarray(np.asarray(xs), dtype=np.float32)
    ys = np.ascontiguousarray(np.asarray(ys), dtype=np.float32)
    assert xs.shape == (16, L, D) and ys.shape == (16, L, D)
    nc = _get_nc()
    idn = np.eye(L, dtype=np.float32)
    shf = np.eye(L, k=-1, dtype=np.float32)
    msk = (np.arange(L) % 4 == 0).astype(np.int32).reshape(L, 1)
    in_maps = []
    for c in range(N_CORES):
        in_maps.append(
            {
                "xs": xs[2 * c:2 * c + 2].reshape(NX * L, D).copy(),
                "ys": ys.reshape(NY * L, D).copy(),
                "idn": idn,
                "shf": shf,
                "msk": msk,
            }
        )
    try:
        res = run_bass_kernel_spmd(nc, in_maps, list(range(N_CORES)), trace=trace)
    except ModuleNotFoundError:
        res = run_bass_kernel_spmd(nc, in_maps, list(range(N_CORES)), trace=False)
    rows = [res.results[c]["out"].reshape(NX, NY) for c in range(N_CORES)]
    out = np.concatenate(rows, axis=0)
    return out, res


def kernel(xs, ys):
    out, _ = run(xs, ys)
    return out
